# revision 1
# baseline (speedup 1.0000x reference)
"""Trainium2 Bass kernel for nn_CIN (xDeepFM compressed-interaction network).

Math: each CIN layer computes, per sample b and feature-dim d (a "column"
n=(b,d)):  y[o] = sum_{h,m} W[o,h,m] * a[h] * b[m]  — a bilinear form.

We avoid materializing the outer-product tensor z[h*m, n] (which needs slow
cross-partition broadcasts) by polarization:  a*b = ((a+b)^2 - a^2 - b^2)/2.
Each layer becomes:  s = V @ t   (pair sums, TensorE)
                     q = s*s     (elementwise square, ScalarE/VectorE)
                     y = C @ q + G @ t^2   (TensorE, PSUM-accumulated)
with V a 0/1 pair-selection matrix and C,G folded from W host-side (exact).

Layer 0 uses the symmetric fold (741 unordered pairs of 39 features);
layer 1 uses all 64*39=2496 (nh,x) pairs.  Everything on-device is fp16
(inputs/weights) with fp32 PSUM accumulation.

Sharding: pure data parallel — batch 4096 split as 512 per NeuronCore
across 8 cores; weights replicated.
"""

import numpy as np

B, F, D = 4096, 39, 16
L0, L1 = 128, 128
H1 = L0 // 2                      # 64 hidden maps feed layer 1
NCORES = 8
BL = B // NCORES                  # 512 samples per core
NCOL = BL * D                     # 8192 columns per core
NT = 512                          # columns per tile
NTILES = NCOL // NT               # 16
NB = NT // D                      # samples per tile (32)

K0 = F * (F - 1) // 2             # 741 layer-0 pairs
K1 = H1 * F                       # 2496 layer-1 pairs
T0 = F                            # t rows for layer 0 rhs (x)
T1 = 128                          # t rows: [x 0:39 | zeros 39:64 | nh 64:128]
NH0 = 64                          # nh base partition in t


def _chunks(k):
    out = []
    o = 0
    while o < k:
        c = min(128, k - o)
        out.append((o, c))
        o += k
        o = out[-1][0] + c
    return out


CH0 = _chunks(K0)                 # [(0,128)x5, (640,101)]
CH1 = _chunks(K1)                 # [(0,128)x19, (2432,64)]
NC0 = len(CH0)
NC1 = len(CH1)


def _host_weights(W0, b0, W1, b1):
    """Fold W0/W1 into the square-trick operands (all exact, fp32)."""
    W0 = np.asarray(W0, np.float32)
    W1 = np.asarray(W1, np.float32)
    S0 = W0.reshape(L0, F, F)
    S0 = (S0 + S0.transpose(0, 2, 1)) / 2
    iu = np.triu_indices(F, 1)                       # 741 (h<m) pairs
    V0 = np.zeros((K0, F), np.float32)
    V0[np.arange(K0), iu[0]] = 1
    V0[np.arange(K0), iu[1]] = 1
    C0 = S0[:, iu[0], iu[1]]                         # [128, 741]
    rowsum = S0.sum(2)
    G0 = np.einsum('ohh->oh', S0) * 2 - rowsum       # S[h,h] - sum_{m!=h} S[h,m]

    B1 = W1.reshape(L1, H1, F)
    hh, mm = np.meshgrid(np.arange(H1), np.arange(F), indexing='ij')
    hh, mm = hh.ravel(), mm.ravel()                  # 2496 pairs, h-major
    V1 = np.zeros((K1, T1), np.float32)
    V1[np.arange(K1), mm] = 1                        # x part at rows 0:39
    V1[np.arange(K1), NH0 + hh] = 1                  # nh part at rows 64:128
    C1 = B1[:, hh, mm] / 2                           # [128, 2496]
    G1 = np.zeros((L1, T1), np.float32)
    G1[:, :F] = -B1.sum(1) / 2                       # coeff on x^2
    G1[:, NH0:] = -B1.sum(2) / 2                     # coeff on nh^2

    def pack_stationary(Ct, chunks):
        # Ct: [K, 128] -> packed [128, 128*nchunks] fp16, chunk i in
        # partitions 0:kc, free cols i*128:(i+1)*128
        out = np.zeros((128, 128 * len(chunks)), np.float16)
        for i, (o, kc) in enumerate(chunks):
            out[:kc, i * 128:i * 128 + 128] = Ct[o:o + kc, :]
        return out

    def pad_cols(Vt, n):
        out = np.zeros((Vt.shape[0], n), np.float16)
        out[:, :Vt.shape[1]] = Vt
        return out

    return {
        "V0T": pad_cols(V0.T, 128 * NC0),            # [39, 768]
        "V1T": pad_cols(V1.T, 128 * NC1),            # [103, 2560]
        "C0T": pack_stationary(C0.T, CH0),           # [128, 768]
        "C1T": pack_stationary(C1.T, CH1),           # [128, 2560]
        "G0T": G0.T.astype(np.float16),              # [39, 128]
        "G1T": G1.T.astype(np.float16),              # [103, 128]
        "b0": np.asarray(b0, np.float32).reshape(L0, 1),
        "b1": np.asarray(b1, np.float32).reshape(L1, 1),
    }


_NC_CACHE = {}


def _build_nc(repeat=1):
    key = ("nc", repeat)
    if key in _NC_CACHE:
        return _NC_CACHE[key]
    from contextlib import ExitStack
    import concourse.bacc as bacc
    import concourse.mybir as mybir
    import concourse.tile as tile

    f16 = mybir.dt.float16
    f32 = mybir.dt.float32

    nc = bacc.Bacc("TRN2", target_bir_lowering=False, debug=False)

    xT_d = nc.dram_tensor("xT", [F, NCOL], f16, kind="ExternalInput")
    V0T_d = nc.dram_tensor("V0T", [F, 128 * NC0], f16, kind="ExternalInput")
    V1T_d = nc.dram_tensor("V1T", [T1, 128 * NC1], f16, kind="ExternalInput")
    C0T_d = nc.dram_tensor("C0T", [128, 128 * NC0], f16, kind="ExternalInput")
    C1T_d = nc.dram_tensor("C1T", [128, 128 * NC1], f16, kind="ExternalInput")
    G0T_d = nc.dram_tensor("G0T", [F, 128], f16, kind="ExternalInput")
    G1T_d = nc.dram_tensor("G1T", [T1, 128], f16, kind="ExternalInput")
    b0_d = nc.dram_tensor("b0", [L0, 1], f32, kind="ExternalInput")
    b1_d = nc.dram_tensor("b1", [L1, 1], f32, kind="ExternalInput")
    out_d = nc.dram_tensor("out", [L0 - H1 + L1, BL], f32, kind="ExternalOutput")

    Relu = mybir.ActivationFunctionType.Relu

    with tile.TileContext(nc) as tc, ExitStack() as ctx:
        const = ctx.enter_context(tc.tile_pool(name="const", bufs=1))
        tp = ctx.enter_context(tc.tile_pool(name="tp", bufs=2))
        t2p = ctx.enter_context(tc.tile_pool(name="t2p", bufs=2))
        sq0p = ctx.enter_context(tc.tile_pool(name="sq0p", bufs=2))
        sq1p = ctx.enter_context(tc.tile_pool(name="sq1p", bufs=2))
        rp = ctx.enter_context(tc.tile_pool(name="rp", bufs=2))
        outp = ctx.enter_context(tc.tile_pool(name="outp", bufs=1))
        sps = ctx.enter_context(tc.tile_pool(name="sps", bufs=4, space="PSUM"))
        yps0 = ctx.enter_context(tc.tile_pool(name="yps0", bufs=2, space="PSUM"))
        yps1 = ctx.enter_context(tc.tile_pool(name="yps1", bufs=2, space="PSUM"))

        # resident weights
        V0T = const.tile([F, 128 * NC0], f16)
        V1T = const.tile([T1, 128 * NC1], f16)
        C0T = const.tile([128, 128 * NC0], f16)
        C1T = const.tile([128, 128 * NC1], f16)
        G0T = const.tile([F, 128], f16)
        G1T = const.tile([T1, 128], f16)
        b0t = const.tile([L0, 1], f32)
        b1t = const.tile([L1, 1], f32)
        for dst, src in ((V0T, V0T_d), (V1T, V1T_d), (C0T, C0T_d),
                         (C1T, C1T_d), (G0T, G0T_d), (G1T, G1T_d),
                         (b0t, b0_d), (b1t, b1_d)):
            nc.sync.dma_start(out=dst[:], in_=src.ap())

        out0 = outp.tile([H1, BL], f32)
        out1 = outp.tile([L1, BL], f32)

        for nt in [nt for _ in range(repeat) for nt in range(NTILES)]:
            csl = slice(nt * NT, (nt + 1) * NT)
            # t = [x (0:39); nh (39:103)]
            t = tp.tile([T1, NT], f16)
            t2 = t2p.tile([T1, NT], f16)
            nc.vector.memset(t[32:NH0, :], 0.0)              # zero pad rows
            nc.vector.memset(t2[32:NH0, :], 0.0)
            nc.sync.dma_start(out=t[0:F, :], in_=xT_d.ap()[:, csl])
            nc.scalar.square(t2[0:F, :], t[0:F, :])          # x^2

            # ---- layer 0: s0 = V0 @ x ; sq0 = s0^2 ----
            sq0 = sq0p.tile([128, NC0 * NT], f16)
            for i, (o, kc) in enumerate(CH0):
                ps = sps.tile([128, NT], f32)
                nc.tensor.matmul(ps[0:kc, :], V0T[:, i * 128:i * 128 + kc],
                                 t[0:F, :], start=True, stop=True)
                dst = sq0[0:kc, i * NT:(i + 1) * NT]
                if i % 5 in (1, 3):
                    nc.vector.tensor_copy(dst, ps[0:kc, :])
                    nc.vector.tensor_mul(dst, dst, dst)
                else:
                    nc.scalar.square(dst, ps[0:kc, :])

            # ---- y0 = C0 @ sq0 + G0 @ x^2 ----
            y0 = yps0.tile([L0, NT], f32)
            for i, (o, kc) in enumerate(CH0):
                nc.tensor.matmul(y0[:], C0T[0:kc, i * 128:(i + 1) * 128],
                                 sq0[0:kc, i * NT:(i + 1) * NT],
                                 start=(i == 0), stop=False)
            nc.tensor.matmul(y0[:], G0T[:], t2[0:F, :], start=False, stop=True)

            # relu + split
            nc.scalar.activation(t[NH0:T1, :], y0[0:H1, :], Relu, bias=b0t[0:H1])
            r0 = rp.tile([H1, NT], f32, tag="r0")
            nc.scalar.activation(r0[:], y0[H1:L0, :], Relu, bias=b0t[H1:L0])
            nc.scalar.square(t2[NH0:T1, :], t[NH0:T1, :])    # nh^2

            # ---- layer 1: s1 = V1 @ [x; nh] ; sq1 = s1^2 ----
            sq1 = sq1p.tile([128, NC1 * NT], f16)
            for i, (o, kc) in enumerate(CH1):
                ps = sps.tile([128, NT], f32)
                nc.tensor.matmul(ps[0:kc, :], V1T[:, i * 128:i * 128 + kc],
                                 t[:], start=True, stop=True)
                dst = sq1[0:kc, i * NT:(i + 1) * NT]
                if i % 5 in (1, 3):
                    nc.vector.tensor_copy(dst, ps[0:kc, :])
                    nc.vector.tensor_mul(dst, dst, dst)
                else:
                    nc.scalar.square(dst, ps[0:kc, :])

            # ---- y1 = C1 @ sq1 + G1 @ t^2 ----
            y1 = yps1.tile([L1, NT], f32)
            for i, (o, kc) in enumerate(CH1):
                nc.tensor.matmul(y1[:], C1T[0:kc, i * 128:(i + 1) * 128],
                                 sq1[0:kc, i * NT:(i + 1) * NT],
                                 start=(i == 0), stop=False)
            nc.tensor.matmul(y1[:], G1T[:], t2[:], start=False, stop=True)

            r1 = rp.tile([L1, NT], f32, tag="r1")
            nc.scalar.activation(r1[:], y1[:], Relu, bias=b1t[:])

            # ---- sum over d (innermost 16 of each column group) ----
            bsl = slice(nt * NB, (nt + 1) * NB)
            nc.vector.tensor_reduce(
                out0[:, bsl], r0[:].rearrange("p (b d) -> p b d", d=D),
                axis=mybir.AxisListType.X, op=mybir.AluOpType.add)
            nc.vector.tensor_reduce(
                out1[:, bsl], r1[:].rearrange("p (b d) -> p b d", d=D),
                axis=mybir.AxisListType.X, op=mybir.AluOpType.add)

        nc.sync.dma_start(out=out_d.ap()[0:H1, :], in_=out0[:])
        nc.sync.dma_start(out=out_d.ap()[H1:, :], in_=out1[:])

    nc.compile()
    _NC_CACHE[key] = nc
    return nc


def _run(inputs, trace=False):
    from concourse.bass_utils import run_bass_kernel_spmd

    x = np.asarray(inputs["x"], np.float32)
    w = _host_weights(inputs["W0"], inputs["b0"], inputs["W1"], inputs["b1"])
    nc = _build_nc()

    in_maps = []
    for c in range(NCORES):
        xs = x[c * BL:(c + 1) * BL]                          # [512, 39, 16]
        xT = np.ascontiguousarray(
            xs.transpose(1, 0, 2).reshape(F, NCOL)).astype(np.float16)
        m = {"xT": xT}
        m.update(w)
        in_maps.append(m)

    res = run_bass_kernel_spmd(nc, in_maps, core_ids=list(range(NCORES)),
                               trace=trace)
    out = np.empty((B, L0 - H1 + L1), np.float32)
    for c in range(NCORES):
        out[c * BL:(c + 1) * BL] = res.results[c]["out"].T
    return out, res


def kernel(**inputs):
    out, _ = _run(inputs)
    return out

